# revision 20
# baseline (speedup 1.0000x reference)
"""Multi-head attention (B=4, S=2048, D=1024, H=16, causal) on 8 TRN2 NeuronCores.

Sharding: core c -> (batch b = c//2, head-group hg = c%2 of 8 heads).
Per core: QKV projections for its 8 heads (via on-chip transpose of x),
causal attention in transposed layout (scoresT[t, s]), softmax denominator
via 64 appended ones-columns in the att@V matmul (denominator replicated
into po rows 64..127 for free), approx reciprocal + rescale, then a
row-parallel output projection producing a partial [S, D]. Host sums the
two head-group partials per batch and adds the bias.

Precision: bf16 operands for x/Wqkv/Q/K/V/exp-weights (fp32 PSUM
accumulation), f32r for the output projection. Scores of the two heads of
a pair are issued back-to-back at K=64 so they run concurrently in
disjoint PE row-groups. Next pair's QKV/Vp work is emitted after this
pair's attention so the Tile scheduler uses it to fill PE bubbles while
the scalar engine works through the exps (keeps HAM at full clock).
"""

import sys

import numpy as np

for _p in ("/opt/trn_rl_repo", "/root/.axon_site/_ro/trn_rl_repo"):
    if _p not in sys.path:
        sys.path.append(_p)

import ml_dtypes

import concourse.bass as bass
import concourse.tile as tile
from concourse import mybir
from concourse.bass_utils import run_bass_kernel_spmd

F32 = mybir.dt.float32
I32 = mybir.dt.int32
F32R = mybir.dt.float32r
BF16 = mybir.dt.bfloat16

B, S, D, H, HD = 4, 2048, 1024, 16, 64
P = 128
NPAIR = 4  # head pairs per core (8 heads)
NS = S // 512  # 4 s-runs of 512
NST = S // P  # 16 s-tiles of 128
NDC = D // P  # 8 d-chunks

_WAIT_EXEMPT = {
    "InstEventSemaphore",
    "InstUnconditionalBranch",
    "InstCall",
    "InstRegisterMove",
}


def fix_extra_waits(nc):
    """TRN2 compute-instruction structs encode at most one semaphore wait.
    After Tile scheduling, move extra waits onto engine NOPs inserted just
    before the over-constrained instruction (same engine, final order)."""
    import copy

    templates = {}

    def make_nop(engine):
        if engine not in templates:
            nc.engines[engine].nop()
            tail = nc.m.functions[0].blocks[-1]
            insts = tail.instructions
            templates[engine] = insts.pop()
            tail.instructions = insts
        nop = copy.deepcopy(templates[engine])
        nop.name = nc.get_next_instruction_name()
        return nop

    n_fixed = 0
    for fn in nc.m.functions:
        for blk in fn.blocks:
            out = []
            for inst in blk.instructions:
                si = getattr(inst, "sync_info", None)
                if (
                    type(inst).__name__ not in _WAIT_EXEMPT
                    and si is not None
                    and si.on_wait
                    and len(si.on_wait) > 1
                ):
                    waits = list(si.on_wait)
                    for w in waits[:-1]:
                        nop = make_nop(inst.engine)
                        nop.sync_info = mybir.SyncInfo(on_wait=[w], on_update=[])
                        out.append(nop)
                    si.on_wait = [waits[-1]]
                    n_fixed += 1
                out.append(inst)
            blk.instructions = out
    return n_fixed


def build_nc():
    nc = bass.Bass()
    xt_d = nc.dram_tensor("xt", [NDC, P, NS, 512], BF16, kind="ExternalInput")
    wq_d = nc.dram_tensor("wq", [NPAIR, P, NDC, P], BF16, kind="ExternalInput")
    wk_d = nc.dram_tensor("wk", [NPAIR, P, NDC, P], BF16, kind="ExternalInput")
    wv_d = nc.dram_tensor("wv", [NPAIR, P, NDC, P], BF16, kind="ExternalInput")
    wp_d = nc.dram_tensor("wp", [NPAIR, P, D], F32, kind="ExternalInput")
    ident_d = nc.dram_tensor("ident", [P, P], BF16, kind="ExternalInput")
    trimask_d = nc.dram_tensor("trimask", [P, P], F32, kind="ExternalInput")
    ones_d = nc.dram_tensor("ones", [P, P], F32, kind="ExternalInput")
    y_d = nc.dram_tensor("y", [S, D], F32, kind="ExternalOutput")

    with tile.TileContext(nc) as tc:
        with (
            tc.tile_pool(name="consts", bufs=1) as consts,
            tc.tile_pool(name="pw", bufs=2) as pw,
            tc.tile_pool(name="pqk", bufs=2) as pqk,
            tc.tile_pool(name="pex", bufs=4) as pex,
            tc.tile_pool(name="pio", bufs=2) as pio,
            tc.tile_pool(name="psS", bufs=2, space="PSUM") as psS,
            tc.tile_pool(name="psO", bufs=1, space="PSUM") as psO,
            tc.tile_pool(name="psM", bufs=2, space="PSUM") as psM,
        ):
            ident = consts.tile([P, P], BF16, tag="ident")
            nc.sync.dma_start(ident, ident_d[:, :])
            trimask = consts.tile([P, P], F32, tag="trimask")
            nc.sync.dma_start(trimask, trimask_d[:, :])
            # x transposed: [d-part, d-chunk, s]
            xT = consts.tile([P, NDC, S], BF16, tag="xT")
            # normalized attention output, transposed: [pair-hk part, pair, s]
            OcatT = consts.tile([P, NPAIR, S], F32R, tag="OcatT")
            wp_sb = consts.tile([P, NPAIR, D], F32R, tag="wp")
            for p in range(NPAIR):
                nc.sync.dma_start(wp_sb[:, p, :], wp_d[p].bitcast(F32R))
            # V natural with ones-halves: [t-part, t-tile, head, (V 64 | 1 64)]
            # two persistent buffers, ping-ponged across pairs
            vps = []
            for i in range(2):
                vp_i = consts.tile([P, NST, 2, P], BF16, tag=f"vp{i}", name=f"vp{i}")
                nc.gpsimd.memset(vp_i[:, :, :, HD:P], 1.0)
                vps.append(vp_i)
            ones1 = consts.tile([1, HD], F32R, tag="ones1")
            nc.sync.dma_start(ones1, ones_d[0:1, 0:HD].bitcast(F32R))

            def load_pair_weights(p):
                ws = {}
                for nm, dram in (("wq", wq_d), ("wk", wk_d), ("wv", wv_d)):
                    w_sb = pw.tile([P, NDC, P], BF16, tag=nm, name=nm)
                    nc.sync.dma_start(w_sb, dram[p])
                    ws[nm] = w_sb
                return ws

            def emit_qkv_run(ws, qkv, sc_run):
                s0 = sc_run * 512
                for nm, dst in (("wq", qkv["QT"]), ("wk", qkv["KT"]), ("wv", qkv["VT"])):
                    ps = psM.tile([P, 512], F32, tag="mm", name="mm")
                    for dc in range(NDC):
                        nc.tensor.matmul(
                            ps,
                            ws[nm][:, dc],
                            xT[:, dc, s0 : s0 + 512],
                            start=(dc == 0),
                            stop=(dc == NDC - 1),
                        )
                    nc.vector.tensor_copy(out=dst[:, s0 : s0 + 512], in_=ps)

            def emit_vp_tile(qkv, vp, tt):
                pt = psM.tile([P, 512], F32, tag="mm", name="mm")
                ptv = pt.bitcast(BF16)[:, 0:P]
                nc.tensor.transpose(
                    ptv, qkv["VT"][:, tt * P : (tt + 1) * P], ident
                )
                nc.vector.tensor_copy(
                    out=vp[:, tt, :, 0:HD],
                    in_=ptv.rearrange("p (two k) -> p two k", two=2),
                )

            def new_qkv_tiles():
                QT = pqk.tile([P, S], F32R, tag="QT", name="QT")
                KT = pqk.tile([P, S], F32R, tag="KT", name="KT")
                VT = pqk.tile([P, S], BF16, tag="VT", name="VT")
                return {"QT": QT, "KT": KT, "VT": VT}

            # ---- P0: host-pretransposed x streams straight into xT;
            # pair-0 QKV follows per s-run ----
            ws0 = load_pair_weights(0)
            qkv_cur = new_qkv_tiles()
            for sc_run in range(NS):
                for dc in range(NDC):
                    nc.sync.dma_start(
                        xT[:, dc, sc_run * 512 : (sc_run + 1) * 512],
                        xt_d[dc, :, sc_run],
                    )
            for sc_run in range(NS):
                emit_qkv_run(ws0, qkv_cur, sc_run)
            vp_cur = vps[0]
            for tt in range(NST):
                emit_vp_tile(qkv_cur, vp_cur, tt)

            def qkv_filler_units(ws, qkv, vp):
                """Next pair's QKV projections + Vp build as ~40 small PE
                work units, popped one per attention tile to keep the PE
                dense while the scalar engine works through the exps."""
                units = []
                for sc_run in range(NS):
                    s0 = sc_run * 512
                    for nm, dst in (
                        ("wq", qkv["QT"]),
                        ("wk", qkv["KT"]),
                        ("wv", qkv["VT"]),
                    ):
                        holder = {}

                        def unit_a(nm=nm, s0=s0, holder=holder):
                            holder["ps"] = psM.tile(
                                [P, 512], F32, tag="mm", name="mm"
                            )
                            for dc in range(4):
                                nc.tensor.matmul(
                                    holder["ps"],
                                    ws[nm][:, dc],
                                    xT[:, dc, s0 : s0 + 512],
                                    start=(dc == 0),
                                    stop=False,
                                )

                        def unit_b(nm=nm, dst=dst, s0=s0, holder=holder):
                            ps = holder["ps"]
                            for dc in range(4, NDC):
                                nc.tensor.matmul(
                                    ps,
                                    ws[nm][:, dc],
                                    xT[:, dc, s0 : s0 + 512],
                                    start=False,
                                    stop=(dc == NDC - 1),
                                )
                            nc.vector.tensor_copy(
                                out=dst[:, s0 : s0 + 512], in_=ps
                            )

                        units.append(unit_a)
                        units.append(unit_b)
                    for tt in range(4 * sc_run, 4 * sc_run + 4):
                        units.append(
                            lambda tt=tt: emit_vp_tile(qkv, vp, tt)
                        )
                return units

            def outproj_units(sts):
                units = []
                for st in sts:
                    for half in (0, 1):

                        def unit(st=st, half=half):
                            psy = psM.tile([P, 512], F32, tag="mm", name="mm")
                            for pp in range(NPAIR):
                                nc.tensor.matmul(
                                    psy,
                                    OcatT[:, pp, st * P : (st + 1) * P],
                                    wp_sb[:, pp, half * 512 : (half + 1) * 512],
                                    start=(pp == 0),
                                    stop=(pp == NPAIR - 1),
                                )
                            yt = pio.tile([P, 512], F32, tag="yt", name="yt")
                            nc.vector.tensor_copy(out=yt, in_=psy)
                            nc.sync.dma_start(
                                y_d[
                                    st * P : (st + 1) * P,
                                    half * 512 : (half + 1) * 512,
                                ],
                                yt,
                            )

                        units.append(unit)
                return units

            from collections import deque

            fill_q = deque()

            # ---- attention per pair, software-pipelined: att@V lags the
            # scores by 2 tiles, one filler unit per tile, PSUM freed via a
            # single [65,512] copy so the softmax normalize (ln/exp on the
            # scalar engine + PE ones-broadcast) is fully off-critical-path.
            for p in range(NPAIR):
                qkv, vp = qkv_cur, vp_cur
                if p < NPAIR - 1:
                    ws_next = load_pair_weights(p + 1)
                    qkv_cur = new_qkv_tiles()
                    vp_cur = vps[(p + 1) % 2]
                    fill_q.extend(qkv_filler_units(ws_next, qkv_cur, vp_cur))
                pending_norm = None
                for sr in range(NS):
                    s0 = sr * 512
                    n_t = 4 * (sr + 1)
                    po = [
                        psO.tile([P, 512], F32, tag=f"po{h}", name=f"po{h}")
                        for h in (0, 1)
                    ]
                    ets = {}

                    def emit_av(tt, sr=sr, n_t=n_t, po=po, ets=ets, vp=vp):
                        j = tt - 4 * sr
                        off = P * j if j >= 1 else 0
                        et = ets.pop(tt)
                        for h in (0, 1):
                            nc.tensor.matmul(
                                po[h][:, off:],
                                vp[:, tt, h, :],
                                et[:, h, off:],
                                start=(tt == 0),
                                stop=(tt == n_t - 1),
                                skip_group_check=True,
                            )

                    for tt in range(n_t):
                        j = tt - 4 * sr
                        pss = psS.tile([P, 2, 512], F32, tag="pss", name="pss")
                        for h in (0, 1):
                            nc.tensor.matmul(
                                pss[:, h],
                                qkv["KT"][64 * h : 64 * h + 64, tt * P : (tt + 1) * P],
                                qkv["QT"][64 * h : 64 * h + 64, s0 : s0 + 512],
                                start=True,
                                stop=True,
                            )
                        if j >= 0:
                            for h in (0, 1):
                                nc.vector.tensor_tensor(
                                    pss[:, h, P * j : P * (j + 1)],
                                    pss[:, h, P * j : P * (j + 1)],
                                    trimask,
                                    mybir.AluOpType.add,
                                )
                        et = pex.tile([P, 2, 512], BF16, tag="et", name="et")
                        nc.scalar.activation(
                            out=et,
                            in_=pss,
                            func=mybir.ActivationFunctionType.Exp,
                            scale=float(HD**-0.5),
                        )
                        ets[tt] = et
                        if tt == 1 and pending_norm is not None:
                            pending_norm()
                            pending_norm = None
                        if tt >= 2:
                            emit_av(tt - 2)
                        if fill_q:
                            fill_q.popleft()()
                    emit_av(n_t - 2)
                    if fill_q:
                        fill_q.popleft()()
                    emit_av(n_t - 1)
                    # free po banks early: one copy grabs numerators + denom
                    onums = []
                    for h in (0, 1):
                        onum = pex.tile(
                            [HD + 1, 512], F32, tag="onum", bufs=4, name="onum"
                        )
                        nc.vector.tensor_copy(out=onum, in_=po[h][0 : HD + 1, :])
                        onums.append(onum)

                    def norm(onums=onums, p=p, s0=s0, sr=sr):
                        for h in (0, 1):
                            onum = onums[h]
                            lnd = pex.tile(
                                [1, 512], F32, tag="lnd", bufs=2, name="lnd"
                            )
                            nc.scalar.activation(
                                out=lnd,
                                in_=onum[HD : HD + 1, :],
                                func=mybir.ActivationFunctionType.Ln,
                            )
                            rcp = pex.tile(
                                [1, 512], F32R, tag="rcp", bufs=2, name="rcp"
                            )
                            nc.scalar.activation(
                                out=rcp,
                                in_=lnd,
                                func=mybir.ActivationFunctionType.Exp,
                                scale=-1.0,
                            )
                            pb = psM.tile([P, 512], F32, tag="mm", name="mm")
                            nc.tensor.matmul(
                                pb[0:HD, :], ones1, rcp, start=True, stop=True
                            )
                            rb = pex.tile(
                                [HD, 512], F32, tag="rb", bufs=2, name="rb"
                            )
                            nc.vector.tensor_copy(out=rb, in_=pb[0:HD, :])
                            nc.vector.tensor_tensor(
                                OcatT[64 * h : 64 * h + 64, p, s0 : s0 + 512],
                                onum[0:HD, :],
                                rb,
                                mybir.AluOpType.mult,
                            )
                        if p == NPAIR - 1:
                            fill_q.extend(
                                outproj_units(range(4 * sr, 4 * sr + 4))
                            )

                    pending_norm = norm
                if pending_norm is not None:
                    pending_norm()
            # drain remaining filler (last runs' output projection)
            while fill_q:
                fill_q.popleft()()

    fix_extra_waits(nc)
    return nc


_NC = None


def _get_nc():
    global _NC
    if _NC is None:
        _NC = build_nc()
    return _NC


def _prep_core_inputs(x, Wq, Wk, Wv, Wp, core):
    b, hg = core // 2, core % 2
    hsl = slice(hg * 8, hg * 8 + 8)
    bf = ml_dtypes.bfloat16

    def prep_w(W):
        # [8, D, HD] -> [pair, dp, dc, (hi k)]
        a = W[hsl].reshape(NPAIR, 2, NDC, P, HD)
        return np.ascontiguousarray(
            a.transpose(0, 3, 2, 1, 4).reshape(NPAIR, P, NDC, P)
        ).astype(bf)

    return {
        "xt": np.ascontiguousarray(x[b].T).astype(bf).reshape(NDC, P, NS, 512),
        "wq": prep_w(Wq),
        "wk": prep_w(Wk),
        "wv": prep_w(Wv),
        "wp": np.ascontiguousarray(
            Wp[hg * 512 : (hg + 1) * 512].reshape(NPAIR, P, D)
        ),
        "ident": np.eye(P, dtype=np.float32).astype(bf),
        "ones": np.ones((P, P), dtype=np.float32),
        "trimask": np.where(
            np.arange(P)[None, :] >= np.arange(P)[:, None], 0.0, -1e30
        ).astype(np.float32),
    }


def kernel(trace=False, **inputs):
    x = np.asarray(inputs["x"], dtype=np.float32)
    Wq = np.asarray(inputs["Wq"], dtype=np.float32)
    Wk = np.asarray(inputs["Wk"], dtype=np.float32)
    Wv = np.asarray(inputs["Wv"], dtype=np.float32)
    Wp = np.asarray(inputs["Wp"], dtype=np.float32)
    bp = np.asarray(inputs["bp"], dtype=np.float32)

    nc = _get_nc()
    in_maps = [_prep_core_inputs(x, Wq, Wk, Wv, Wp, c) for c in range(8)]
    res = run_bass_kernel_spmd(nc, in_maps, core_ids=list(range(8)), trace=trace)

    out = np.empty((B, S, D), dtype=np.float32)
    for b in range(B):
        out[b] = res.results[2 * b]["y"] + res.results[2 * b + 1]["y"] + bp
    if trace:
        return out, res
    return out


# revision 22
# speedup vs baseline: 1.2224x; 1.2224x over previous
"""Multi-head attention (B=4, S=2048, D=1024, H=16, causal) on 8 TRN2 NeuronCores.

Sharding: core c -> (batch b = c//2, head-group hg = c%2 of 8 heads).
Per core: QKV projections for its 8 heads (via on-chip transpose of x),
causal attention in transposed layout (scoresT[t, s]), softmax denominator
via 64 appended ones-columns in the att@V matmul (denominator replicated
into po rows 64..127 for free), approx reciprocal + rescale, then a
row-parallel output projection producing a partial [S, D]. Host sums the
two head-group partials per batch and adds the bias.

Precision: bf16 operands for x/Wqkv/Q/K/V/exp-weights (fp32 PSUM
accumulation), f32r for the output projection. Scores of the two heads of
a pair are issued back-to-back at K=64 so they run concurrently in
disjoint PE row-groups. Next pair's QKV/Vp work is emitted after this
pair's attention so the Tile scheduler uses it to fill PE bubbles while
the scalar engine works through the exps (keeps HAM at full clock).
"""

import sys

import numpy as np

for _p in ("/opt/trn_rl_repo", "/root/.axon_site/_ro/trn_rl_repo"):
    if _p not in sys.path:
        sys.path.append(_p)

import ml_dtypes

import concourse.bass as bass
import concourse.tile as tile
from concourse import mybir
from concourse.bass_utils import run_bass_kernel_spmd

F32 = mybir.dt.float32
I32 = mybir.dt.int32
F32R = mybir.dt.float32r
BF16 = mybir.dt.bfloat16

B, S, D, H, HD = 4, 2048, 1024, 16, 64
P = 128
NPAIR = 4  # head pairs per core (8 heads)
NS = S // 512  # 4 s-runs of 512
NST = S // P  # 16 s-tiles of 128
NDC = D // P  # 8 d-chunks

_WAIT_EXEMPT = {
    "InstEventSemaphore",
    "InstUnconditionalBranch",
    "InstCall",
    "InstRegisterMove",
}


def fix_extra_waits(nc):
    """TRN2 compute-instruction structs encode at most one semaphore wait.
    After Tile scheduling, move extra waits onto engine NOPs inserted just
    before the over-constrained instruction (same engine, final order)."""
    import copy

    templates = {}

    def make_nop(engine):
        if engine not in templates:
            nc.engines[engine].nop()
            tail = nc.m.functions[0].blocks[-1]
            insts = tail.instructions
            templates[engine] = insts.pop()
            tail.instructions = insts
        nop = copy.deepcopy(templates[engine])
        nop.name = nc.get_next_instruction_name()
        return nop

    n_fixed = 0
    for fn in nc.m.functions:
        for blk in fn.blocks:
            out = []
            for inst in blk.instructions:
                si = getattr(inst, "sync_info", None)
                if (
                    type(inst).__name__ not in _WAIT_EXEMPT
                    and si is not None
                    and si.on_wait
                    and len(si.on_wait) > 1
                ):
                    waits = list(si.on_wait)
                    for w in waits[:-1]:
                        nop = make_nop(inst.engine)
                        nop.sync_info = mybir.SyncInfo(on_wait=[w], on_update=[])
                        out.append(nop)
                    si.on_wait = [waits[-1]]
                    n_fixed += 1
                out.append(inst)
            blk.instructions = out
    return n_fixed


def build_nc():
    nc = bass.Bass()
    xt_d = nc.dram_tensor("xt", [NDC, P, NS, 512], BF16, kind="ExternalInput")
    wq_d = nc.dram_tensor("wq", [NPAIR, P, NDC, P], BF16, kind="ExternalInput")
    wk_d = nc.dram_tensor("wk", [NPAIR, P, NDC, P], BF16, kind="ExternalInput")
    wv_d = nc.dram_tensor("wv", [NPAIR, P, NDC, P], BF16, kind="ExternalInput")
    wp_d = nc.dram_tensor("wp", [NPAIR, P, D], F32, kind="ExternalInput")
    ident_d = nc.dram_tensor("ident", [P, P], BF16, kind="ExternalInput")
    trimask_d = nc.dram_tensor("trimask", [P, P], F32, kind="ExternalInput")
    ones_d = nc.dram_tensor("ones", [P, P], F32, kind="ExternalInput")
    y_d = nc.dram_tensor("y", [S, D], F32, kind="ExternalOutput")

    with tile.TileContext(nc) as tc:
        with (
            tc.tile_pool(name="consts", bufs=1) as consts,
            tc.tile_pool(name="pw", bufs=2) as pw,
            tc.tile_pool(name="pqk", bufs=2) as pqk,
            tc.tile_pool(name="pex", bufs=4) as pex,
            tc.tile_pool(name="pio", bufs=2) as pio,
            tc.tile_pool(name="psS", bufs=2, space="PSUM") as psS,
            tc.tile_pool(name="psO", bufs=1, space="PSUM") as psO,
            tc.tile_pool(name="psM", bufs=2, space="PSUM") as psM,
        ):
            ident = consts.tile([P, P], BF16, tag="ident")
            nc.sync.dma_start(ident, ident_d[:, :])
            trimask = consts.tile([P, P], F32, tag="trimask")
            nc.sync.dma_start(trimask, trimask_d[:, :])
            # x transposed: [d-part, d-chunk, s]
            xT = consts.tile([P, NDC, S], BF16, tag="xT")
            # normalized attention output, transposed: [pair-hk part, pair, s]
            OcatT = consts.tile([P, NPAIR, S], F32R, tag="OcatT")
            wp_sb = consts.tile([P, NPAIR, D], F32R, tag="wp")
            for p in range(NPAIR):
                nc.sync.dma_start(wp_sb[:, p, :], wp_d[p].bitcast(F32R))
            # V natural with ones-halves: [t-part, t-tile, head, (V 64 | 1 64)]
            # two persistent buffers, ping-ponged across pairs
            vps = []
            for i in range(2):
                vp_i = consts.tile([P, NST, 2, P], BF16, tag=f"vp{i}", name=f"vp{i}")
                nc.gpsimd.memset(vp_i[:, :, :, HD:P], 1.0)
                vps.append(vp_i)
            ones1 = consts.tile([1, HD], F32R, tag="ones1")
            nc.sync.dma_start(ones1, ones_d[0:1, 0:HD].bitcast(F32R))

            def load_pair_weights(p):
                ws = {}
                for nm, dram in (("wq", wq_d), ("wk", wk_d), ("wv", wv_d)):
                    w_sb = pw.tile([P, NDC, P], BF16, tag=nm, name=nm)
                    nc.sync.dma_start(w_sb, dram[p])
                    ws[nm] = w_sb
                return ws

            def emit_qkv_run(ws, qkv, sc_run):
                s0 = sc_run * 512
                for nm, dst in (("wq", qkv["QT"]), ("wk", qkv["KT"]), ("wv", qkv["VT"])):
                    ps = psM.tile([P, 512], F32, tag="mm", name="mm")
                    for dc in range(NDC):
                        nc.tensor.matmul(
                            ps,
                            ws[nm][:, dc],
                            xT[:, dc, s0 : s0 + 512],
                            start=(dc == 0),
                            stop=(dc == NDC - 1),
                        )
                    nc.vector.tensor_copy(out=dst[:, s0 : s0 + 512], in_=ps)

            def emit_vp_tile(qkv, vp, tt):
                pt = psM.tile([P, 512], F32, tag="mm", name="mm")
                ptv = pt.bitcast(BF16)[:, 0:P]
                nc.tensor.transpose(
                    ptv, qkv["VT"][:, tt * P : (tt + 1) * P], ident
                )
                nc.vector.tensor_copy(
                    out=vp[:, tt, :, 0:HD],
                    in_=ptv.rearrange("p (two k) -> p two k", two=2),
                )

            def new_qkv_tiles():
                QT = pqk.tile([P, S], F32R, tag="QT", name="QT")
                KT = pqk.tile([P, S], F32R, tag="KT", name="KT")
                VT = pqk.tile([P, S], BF16, tag="VT", name="VT")
                return {"QT": QT, "KT": KT, "VT": VT}

            # ---- P0: host-pretransposed x streams straight into xT;
            # pair-0 QKV follows per s-run ----
            ws0 = load_pair_weights(0)
            qkv_cur = new_qkv_tiles()
            for sc_run in range(NS):
                for dc in range(NDC):
                    nc.sync.dma_start(
                        xT[:, dc, sc_run * 512 : (sc_run + 1) * 512],
                        xt_d[dc, :, sc_run],
                    )
            for sc_run in range(NS):
                emit_qkv_run(ws0, qkv_cur, sc_run)
            vp_cur = vps[0]
            for tt in range(NST):
                emit_vp_tile(qkv_cur, vp_cur, tt)

            def qkv_filler_units(ws, qkv, vp):
                """Next pair's QKV projections + Vp build as ~40 small PE
                work units, popped one per attention tile to keep the PE
                dense while the scalar engine works through the exps."""
                units = []
                for sc_run in range(NS):
                    s0 = sc_run * 512
                    for nm, dst in (
                        ("wq", qkv["QT"]),
                        ("wk", qkv["KT"]),
                        ("wv", qkv["VT"]),
                    ):
                        holder = {}

                        def unit_a(nm=nm, s0=s0, holder=holder):
                            holder["ps"] = psM.tile(
                                [P, 512], F32, tag="mm", name="mm"
                            )
                            for dc in range(4):
                                nc.tensor.matmul(
                                    holder["ps"],
                                    ws[nm][:, dc],
                                    xT[:, dc, s0 : s0 + 512],
                                    start=(dc == 0),
                                    stop=False,
                                )

                        def unit_b(nm=nm, dst=dst, s0=s0, holder=holder):
                            ps = holder["ps"]
                            for dc in range(4, NDC):
                                nc.tensor.matmul(
                                    ps,
                                    ws[nm][:, dc],
                                    xT[:, dc, s0 : s0 + 512],
                                    start=False,
                                    stop=(dc == NDC - 1),
                                )
                            nc.vector.tensor_copy(
                                out=dst[:, s0 : s0 + 512], in_=ps
                            )

                        units.append(unit_a)
                        units.append(unit_b)
                    for tt in range(4 * sc_run, 4 * sc_run + 4):
                        units.append(
                            lambda tt=tt: emit_vp_tile(qkv, vp, tt)
                        )
                return units

            def outproj_units(sts):
                units = []
                for st in sts:
                    for half in (0, 1):

                        def unit(st=st, half=half):
                            psy = psM.tile([P, 512], F32, tag="mm", name="mm")
                            for pp in range(NPAIR):
                                nc.tensor.matmul(
                                    psy,
                                    OcatT[:, pp, st * P : (st + 1) * P],
                                    wp_sb[:, pp, half * 512 : (half + 1) * 512],
                                    start=(pp == 0),
                                    stop=(pp == NPAIR - 1),
                                )
                            yt = pio.tile([P, 512], F32, tag="yt", name="yt")
                            nc.vector.tensor_copy(out=yt, in_=psy)
                            nc.sync.dma_start(
                                y_d[
                                    st * P : (st + 1) * P,
                                    half * 512 : (half + 1) * 512,
                                ],
                                yt,
                            )

                        units.append(unit)
                return units

            from collections import deque

            fill_q = deque()

            # ---- attention per pair, software-pipelined: att@V lags the
            # scores by 2 tiles, one filler unit per tile, PSUM freed via a
            # single [65,512] copy so the softmax normalize (ln/exp on the
            # scalar engine + PE ones-broadcast) is fully off-critical-path.
            for p in range(NPAIR):
                qkv, vp = qkv_cur, vp_cur
                if p < NPAIR - 1:
                    ws_next = load_pair_weights(p + 1)
                    qkv_cur = new_qkv_tiles()
                    vp_cur = vps[(p + 1) % 2]
                    fill_q.extend(qkv_filler_units(ws_next, qkv_cur, vp_cur))
                pending_norm = None
                for sr in range(NS):
                    s0 = sr * 512
                    n_t = 4 * (sr + 1)
                    po = [
                        psO.tile([P, 512], F32, tag=f"po{h}", name=f"po{h}")
                        for h in (0, 1)
                    ]
                    ets = {}

                    def emit_av(tt, sr=sr, n_t=n_t, po=po, ets=ets, vp=vp):
                        j = tt - 4 * sr
                        off = P * j if j >= 1 else 0
                        et = ets.pop(tt)
                        for h in (0, 1):
                            nc.tensor.matmul(
                                po[h][:, off:],
                                vp[:, tt, h, :],
                                et[:, h, off:],
                                start=(tt == 0),
                                stop=(tt == n_t - 1),
                                skip_group_check=True,
                            )

                    for tt in range(n_t):
                        j = tt - 4 * sr
                        pss = psS.tile([P, 2, 512], F32, tag="pss", name="pss")
                        for h in (0, 1):
                            nc.tensor.matmul(
                                pss[:, h],
                                qkv["KT"][64 * h : 64 * h + 64, tt * P : (tt + 1) * P],
                                qkv["QT"][64 * h : 64 * h + 64, s0 : s0 + 512],
                                start=True,
                                stop=True,
                            )
                        if j >= 0:
                            for h in (0, 1):
                                nc.vector.tensor_tensor(
                                    pss[:, h, P * j : P * (j + 1)],
                                    pss[:, h, P * j : P * (j + 1)],
                                    trimask,
                                    mybir.AluOpType.add,
                                )
                        et = pex.tile([P, 2, 512], BF16, tag="et", name="et")
                        off = P * j if j >= 1 else 0
                        nc.scalar.activation(
                            out=et[:, :, off:],
                            in_=pss[:, :, off:],
                            func=mybir.ActivationFunctionType.Exp,
                            scale=float(HD**-0.5),
                        )
                        ets[tt] = et
                        if tt == 1 and pending_norm is not None:
                            pending_norm()
                            pending_norm = None
                        if tt >= 2:
                            emit_av(tt - 2)
                        if fill_q:
                            fill_q.popleft()()
                    emit_av(n_t - 2)
                    if fill_q:
                        fill_q.popleft()()
                    emit_av(n_t - 1)
                    # free po banks early: one copy per head grabs
                    # numerators + the denominator row
                    onum = pex.tile(
                        [HD + 1, 2, 512], F32, tag="onum", bufs=2, name="onum"
                    )
                    for h in (0, 1):
                        nc.vector.tensor_copy(
                            out=onum[:, h, :], in_=po[h][0 : HD + 1, :]
                        )

                    def norm(onum=onum, p=p, s0=s0, sr=sr):
                        # both heads' 1/denominator in one ln+exp pass
                        lnd = pex.tile(
                            [1, 2, 512], F32, tag="lnd", bufs=2, name="lnd"
                        )
                        nc.scalar.activation(
                            out=lnd,
                            in_=onum[HD : HD + 1, :, :],
                            func=mybir.ActivationFunctionType.Ln,
                        )
                        rcp = pex.tile(
                            [1, 2, 512], F32R, tag="rcp", bufs=2, name="rcp"
                        )
                        nc.scalar.activation(
                            out=rcp,
                            in_=lnd,
                            func=mybir.ActivationFunctionType.Exp,
                            scale=-1.0,
                        )
                        for h in (0, 1):
                            pb = psM.tile([P, 512], F32, tag="mm", name="mm")
                            nc.tensor.matmul(
                                pb[0:HD, :],
                                ones1,
                                rcp[:, h, :],
                                start=True,
                                stop=True,
                            )
                            rb = pex.tile(
                                [HD, 512], F32, tag="rb", bufs=2, name="rb"
                            )
                            nc.vector.tensor_copy(out=rb, in_=pb[0:HD, :])
                            nc.vector.tensor_tensor(
                                OcatT[64 * h : 64 * h + 64, p, s0 : s0 + 512],
                                onum[0:HD, h, :],
                                rb,
                                mybir.AluOpType.mult,
                            )
                        if p == NPAIR - 1:
                            fill_q.extend(
                                outproj_units(range(4 * sr, 4 * sr + 4))
                            )

                    pending_norm = norm
                if pending_norm is not None:
                    pending_norm()
            # drain remaining filler (last runs' output projection)
            while fill_q:
                fill_q.popleft()()

    fix_extra_waits(nc)
    return nc


_NC = None


def _get_nc():
    global _NC
    if _NC is None:
        _NC = build_nc()
    return _NC


def _prep_core_inputs(x, Wq, Wk, Wv, Wp, core):
    b, hg = core // 2, core % 2
    hsl = slice(hg * 8, hg * 8 + 8)
    bf = ml_dtypes.bfloat16

    def prep_w(W):
        # [8, D, HD] -> [pair, dp, dc, (hi k)]
        a = W[hsl].reshape(NPAIR, 2, NDC, P, HD)
        return np.ascontiguousarray(
            a.transpose(0, 3, 2, 1, 4).reshape(NPAIR, P, NDC, P)
        ).astype(bf)

    return {
        "xt": np.ascontiguousarray(x[b].T).astype(bf).reshape(NDC, P, NS, 512),
        "wq": prep_w(Wq),
        "wk": prep_w(Wk),
        "wv": prep_w(Wv),
        "wp": np.ascontiguousarray(
            Wp[hg * 512 : (hg + 1) * 512].reshape(NPAIR, P, D)
        ),
        "ident": np.eye(P, dtype=np.float32).astype(bf),
        "ones": np.ones((P, P), dtype=np.float32),
        "trimask": np.where(
            np.arange(P)[None, :] >= np.arange(P)[:, None], 0.0, -1e30
        ).astype(np.float32),
    }


def kernel(trace=False, **inputs):
    x = np.asarray(inputs["x"], dtype=np.float32)
    Wq = np.asarray(inputs["Wq"], dtype=np.float32)
    Wk = np.asarray(inputs["Wk"], dtype=np.float32)
    Wv = np.asarray(inputs["Wv"], dtype=np.float32)
    Wp = np.asarray(inputs["Wp"], dtype=np.float32)
    bp = np.asarray(inputs["bp"], dtype=np.float32)

    nc = _get_nc()
    in_maps = [_prep_core_inputs(x, Wq, Wk, Wv, Wp, c) for c in range(8)]
    res = run_bass_kernel_spmd(nc, in_maps, core_ids=list(range(8)), trace=trace)

    out = np.empty((B, S, D), dtype=np.float32)
    for b in range(B):
        out[b] = res.results[2 * b]["y"] + res.results[2 * b + 1]["y"] + bp
    if trace:
        return out, res
    return out
